# revision 1
# baseline (speedup 1.0000x reference)
"""Tied-row (MSA) attention, sharded over 8 TRN2 NeuronCores.

Reference computation (b=1, r=128 MSA rows, n=512, 8 heads x 64):
    q, k, v = x @ Wq, x @ Wk, x @ Wv          per-row projections
    dots[h,i,j] = sum_{r,d} q[r,h,i,d] k[r,h,j,d] * scale / sqrt(num_rows)
    attn = softmax_j(dots)                     shared across rows
    out[r,i] = (sum_j attn[h,i,j] v[r,h,j,d]) @ Wo + bo

Sharding: MSA-row axis r split 16-per-core.  Each core computes its partial
logits (reduction over its local r).  Partials are summed with one bf16
AllReduce per head-pair, pipelined behind the following pairs' matmuls;
every core then softmaxes + transposes all heads locally (replicated work,
but no second collective).  Projections, attn @ v and the output projection
are core-local.

x^T / q^T / k^T / v / attn are bf16; out^T and Wo use float32r (full-rate
fp32 matmuls); accumulation is always fp32 in PSUM.
"""

import numpy as np

import concourse.bacc as bacc
import concourse.bass as bass
import concourse.mybir as mybir
import concourse.tile as tile
from concourse import bass_utils
from concourse.masks import make_identity

CORES = 8
R = 16          # MSA rows per core
N = 512         # sequence length
DIM = 256       # model dim
H = 8           # heads
D = 64          # head dim
HD = H * D      # 512
RN = R * N      # 8192 token-rows per core

F32 = mybir.dt.float32
F32R = mybir.dt.float32r
BF16 = mybir.dt.bfloat16

RG = [list(range(CORES))]


def build_nc(scale: float):
    nc = bacc.Bacc(None, target_bir_lowering=False, debug=False)

    x_ext = nc.declare_dram_parameter("x", [RN, DIM], F32, isOutput=False)
    wq_ext = nc.declare_dram_parameter("wq", [DIM, HD], F32, isOutput=False)
    wk_ext = nc.declare_dram_parameter("wk", [DIM, HD], F32, isOutput=False)
    wv_ext = nc.declare_dram_parameter("wv", [DIM, HD], F32, isOutput=False)
    wo_ext = nc.declare_dram_parameter("wo", [HD, DIM], F32, isOutput=False)
    out_ext = nc.declare_dram_parameter("out", [RN, DIM], F32, isOutput=True)

    # alternate PSUM->SBUF copies between DVE and ScalarE so neither gates
    # PSUM-bank recycling
    _cp = [0]

    def cp(out, in_):
        if _cp[0] % 2 == 0:
            nc.vector.tensor_copy(out, in_)
        else:
            nc.scalar.copy(out, in_)
        _cp[0] += 1

    with tile.TileContext(nc) as tc:
        # ---- DRAM bounce buffers: one AllReduce per head-pair ----
        dram = tc.alloc_tile_pool(name="dram", bufs=1, space="DRAM")
        ar_in = [dram.tile([2 * N, N], BF16, tag=f"ar_in{hp}", name=f"ar_in{hp}") for hp in range(4)]
        wu_in = dram.tile([128, 8], BF16, tag="wu_in", name="wu_in")
        wu_out = dram.tile([128, 8], BF16, tag="wu_out", name="wu_out", addr_space="Shared")
        ar_out = [
            dram.tile([2 * N, N], BF16, tag=f"ar_out{hp}", name=f"ar_out{hp}", addr_space="Shared")
            for hp in range(4)
        ]

        # ---- pools (allocated up front; releases must be LIFO) ----
        consts = tc.alloc_tile_pool(name="consts", bufs=1)
        v_pool = tc.alloc_tile_pool(name="v", bufs=R * 4)
        xT_pool = tc.alloc_tile_pool(name="xT", bufs=1)
        xrow_pool = tc.alloc_tile_pool(name="xrow", bufs=8)
        wstage = tc.alloc_tile_pool(name="wstage", bufs=2)

        # first x rows prefetch, ahead of the weight DMAs on the same queues
        first_xrs = []
        for c in range(4):
            xr = xrow_pool.tile([128, DIM], F32, tag="xr")
            nc.sync.dma_start(out=xr[:], in_=x_ext[c * 128:(c + 1) * 128, :])
            first_xrs.append(xr)

        # ---- constants ----
        wq_sb = consts.tile([128, 2, HD], BF16, tag="wq")
        wk_sb = consts.tile([128, 2, HD], BF16, tag="wk")
        wv_sb = consts.tile([128, 2, HD], BF16, tag="wv")
        wo_r = consts.tile([128, 4, DIM], F32R, tag="wor")
        id32 = consts.tile([128, 128], F32, tag="id32")
        idbf = consts.tile([128, 128], BF16, tag="idbf")
        for wext, wsb in ((wq_ext, wq_sb), (wk_ext, wk_sb), (wv_ext, wv_sb)):
            wf = wstage.tile([128, 2, HD], F32, tag="wf")
            nc.sync.dma_start(
                out=wf[:], in_=wext[:, :].rearrange("(k p) n -> p k n", p=128)
            )
            nc.any.tensor_copy(wsb[:], wf[:])
        wof = wstage.tile([128, 4, DIM], F32, tag="wf")
        nc.sync.dma_start(
            out=wof[:], in_=wo_ext[:, :].rearrange("(k p) n -> p k n", p=128)
        )
        nc.any.tensor_copy(wo_r[:], wof[:])
        make_identity(nc, id32[:])
        make_identity(nc, idbf[:])

        # warm up ncfw so the first real AllReduce skips the cold-start lag
        nc.sync.dma_start(out=wu_in[:, :], in_=idbf[:, 0:8])
        nc.gpsimd.collective_compute(
            "AllReduce",
            mybir.AluOpType.add,
            replica_groups=RG,
            ins=[wu_in[:, :].opt()],
            outs=[wu_out[:, :].opt()],
        )

        wstage.release()

        xT = xT_pool.tile([128, 2, RN], BF16, tag="xT")

        proj_psum = tc.alloc_tile_pool(name="proj_psum", bufs=3, space="PSUM")
        dots_psum = tc.alloc_tile_pool(name="dots_psum", bufs=3, space="PSUM")
        xp_psum = tc.alloc_tile_pool(name="xp_psum", bufs=2, space="PSUM")

        # ---- load x and transpose to x^T [dim(2x128), rn] (bf16) ----
        # 4 PE transposes batched per PSUM bank -> one [128,512] copy out
        for c4 in range(RN // N):
            xrs = []
            for j in range(4):
                c = c4 * 4 + j
                if c < 4:
                    xr = first_xrs[c]
                else:
                    xr = xrow_pool.tile([128, DIM], F32, tag="xr")
                    nc.sync.dma_start(out=xr[:], in_=x_ext[c * 128:(c + 1) * 128, :])
                xrs.append(xr)
            for kc in range(2):
                pt = xp_psum.tile([128, N], F32, tag="xp")
                for j in range(4):
                    nc.tensor.transpose(
                        pt[:, j * 128:(j + 1) * 128],
                        xrs[j][:, kc * 128:(kc + 1) * 128],
                        id32[:],
                    )
                cp(xT[:, kc, c4 * N:(c4 + 1) * N], pt[:])
        xrow_pool.release()
        xp_psum.release()
        atp_psum = tc.alloc_tile_pool(name="atp_psum", bufs=2, space="PSUM")

        attnT_pool = tc.alloc_tile_pool(name="attnT", bufs=1)
        attnT = attnT_pool.tile([128, H, 4, N], BF16, tag="attnT")
        dstage_pool = tc.alloc_tile_pool(name="dstage", bufs=4)
        smax_pool = tc.alloc_tile_pool(name="smax", bufs=2)
        qkT_pool = tc.alloc_tile_pool(name="qkT", bufs=1)

        def softmax_local(hp):
            """exp/normalize both heads of AllReduce #hp locally, transpose
            into attnT[:, 2hp:2hp+2, :, :]."""
            for m in range(2):
                h = 2 * hp + m
                abfs = []
                for ic in range(4):
                    zt = smax_pool.tile([128, N], BF16, tag="zt")
                    row0 = m * N + ic * 128
                    nc.sync.dma_start(out=zt[:], in_=ar_out[hp][row0:row0 + 128, :])
                    att_f = smax_pool.tile([128, N], F32, tag="att_f")
                    sums = smax_pool.tile([128, 1], F32, tag="sums")
                    nc.scalar.activation(
                        att_f[:],
                        zt[:],
                        mybir.ActivationFunctionType.Exp,
                        scale=scale,
                        accum_out=sums[:],
                    )
                    recip = smax_pool.tile([128, 1], F32, tag="recip")
                    nc.vector.reciprocal(recip[:], sums[:])
                    abf = smax_pool.tile([128, N], BF16, tag="abf", bufs=4)
                    nc.vector.tensor_scalar_mul(abf[:], att_f[:], recip[:])
                    abfs.append(abf)
                for jt in range(4):
                    pt = atp_psum.tile([128, N], BF16, tag="atp")
                    for ic in range(4):
                        nc.tensor.transpose(
                            pt[:, ic * 128:(ic + 1) * 128],
                            abfs[ic][:, jt * 128:(jt + 1) * 128],
                            idbf[:],
                        )
                    cp(attnT[:, h, jt, :], pt[:])

        for hp in range(4):
            qT = qkT_pool.tile([128, RN], BF16, tag="qT")
            kT = qkT_pool.tile([128, RN], BF16, tag="kT")
            for wsb, dstT in ((wq_sb, qT), (wk_sb, kT)):
                for ch in range(RN // N):
                    ps = proj_psum.tile([128, N], F32, tag="proj")
                    for kc in range(2):
                        nc.tensor.matmul(
                            ps[:],
                            wsb[:, kc, hp * 128:(hp + 1) * 128],
                            xT[:, kc, ch * N:(ch + 1) * N],
                            start=(kc == 0),
                            stop=(kc == 1),
                        )
                    cp(dstT[:, ch * N:(ch + 1) * N], ps[:])

            # partial dots for the two heads of this pair; the even head uses
            # PE row-group 0-63, the odd head 64-127 (concurrent row tiles)
            for ic in range(4):
                pe_ = dots_psum.tile([128, N], F32, tag="dots")
                po_ = dots_psum.tile([128, N], F32, tag="dots")
                for rr in range(R):
                    base = rr * N
                    isl = slice(base + ic * 128, base + ic * 128 + 128)
                    jsl = slice(base, base + N)
                    nc.tensor.matmul(
                        pe_[:],
                        qT[0:64, isl],
                        kT[0:64, jsl],
                        start=(rr == 0),
                        stop=(rr == R - 1),
                        skip_group_check=True,
                    )
                    nc.tensor.matmul(
                        po_[:],
                        qT[64:128, isl],
                        kT[64:128, jsl],
                        start=(rr == 0),
                        stop=(rr == R - 1),
                        skip_group_check=True,
                    )
                for m, ps in ((0, pe_), (1, po_)):
                    st = dstage_pool.tile([128, N], BF16, tag="dstage")
                    cp(st[:], ps[:])
                    row0 = m * N + ic * 128
                    nc.sync.dma_start(out=ar_in[hp][row0:row0 + 128, :], in_=st[:])

            nc.gpsimd.collective_compute(
                "AllReduce",
                mybir.AluOpType.add,
                replica_groups=RG,
                ins=[ar_in[hp][:, :].opt()],
                outs=[ar_out[hp][:, :].opt()],
            )

        softmax_local(0)
        qkT_pool.release()

        # ---- v projection (overlaps the AllReduces; reads xT) ----
        v_tiles = {}
        for rr in range(R):
            if rr == 6:
                softmax_local(1)
            for jc in range(4):
                ps = proj_psum.tile([128, N], F32, tag="proj")
                for kc in range(2):
                    nc.tensor.matmul(
                        ps[:],
                        xT[:, kc, rr * N + jc * 128:rr * N + jc * 128 + 128],
                        wv_sb[:, kc, :],
                        start=(kc == 0),
                        stop=(kc == 1),
                    )
                vt = v_pool.tile([128, HD], BF16, tag="v")
                cp(vt[:], ps[:])
                v_tiles[(rr, jc)] = vt

        smax_pool.release()
        dstage_pool.release()
        atp_psum.release()
        dots_psum.release()
        proj_psum.release()

        # ---- attn^T @ v -> out^T, then out @ Wo ----
        # r processed in quarters: all four head-pair blocks for 4 rows, then
        # their output projection; softmax of the last pair lands between the
        # first quarter's hp2 and hp3 blocks so its AllReduce is fully hidden
        oT_pool = tc.alloc_tile_pool(name="oT", bufs=16)
        fstage_pool = tc.alloc_tile_pool(name="fstage", bufs=6)
        av_psum = tc.alloc_tile_pool(name="av_psum", bufs=3, space="PSUM")
        fin_psum = tc.alloc_tile_pool(name="fin_psum", bufs=3, space="PSUM")
        atp2_psum = tc.alloc_tile_pool(name="atp2_psum", bufs=1, space="PSUM")

        def smax_late(hp_l):
            for m in range(2):
                h = 2 * hp_l + m
                abfs = []
                for ic in range(4):
                    zt = fstage_pool.tile([128, N], BF16, tag="zt2", bufs=2)
                    row0 = m * N + ic * 128
                    nc.sync.dma_start(out=zt[:], in_=ar_out[hp_l][row0:row0 + 128, :])
                    att_f = fstage_pool.tile([128, N], F32, tag="att_f2", bufs=2)
                    sums = fstage_pool.tile([128, 1], F32, tag="sums2", bufs=2)
                    nc.scalar.activation(
                        att_f[:],
                        zt[:],
                        mybir.ActivationFunctionType.Exp,
                        scale=scale,
                        accum_out=sums[:],
                    )
                    recip = fstage_pool.tile([128, 1], F32, tag="recip2", bufs=2)
                    nc.vector.reciprocal(recip[:], sums[:])
                    abf = fstage_pool.tile([128, N], BF16, tag="abf2", bufs=4)
                    nc.vector.tensor_scalar_mul(abf[:], att_f[:], recip[:])
                    abfs.append(abf)
                for jt in range(4):
                    pt = atp2_psum.tile([128, N], BF16, tag="atp2")
                    for ic in range(4):
                        nc.tensor.transpose(
                            pt[:, ic * 128:(ic + 1) * 128],
                            abfs[ic][:, jt * 128:(jt + 1) * 128],
                            idbf[:],
                        )
                    cp(attnT[:, h, jt, :], pt[:])

        for rq in range(4):
            oTs = {}
            for hp in range(4):
                if rq == 0 and hp == 2:
                    smax_late(2)
                if rq == 0 and hp == 3:
                    smax_late(3)
                for rx in range(4):
                    rr = rq * 4 + rx
                    ps = av_psum.tile([128, N], F32, tag="av")
                    for jt in range(4):
                        for m in range(2):
                            h = 2 * hp + m
                            nc.tensor.matmul(
                                ps[m * 64:(m + 1) * 64, :],
                                v_tiles[(rr, jt)][:, h * D:(h + 1) * D],
                                attnT[:, h, jt, :],
                                start=(jt == 0),
                                stop=(jt == 3),
                                tile_position=(0, m * 64),
                                skip_group_check=True,
                            )
                    oT = oT_pool.tile([128, N], F32R, tag="oT")
                    cp(oT[:], ps[:])
                    oTs[(rx, hp)] = oT
            for rx in range(4):
                rr = rq * 4 + rx
                for ic in range(4):
                    psf = fin_psum.tile([128, DIM], F32, tag="fin")
                    for kc in range(4):
                        nc.tensor.matmul(
                            psf[:],
                            oTs[(rx, kc)][:, ic * 128:(ic + 1) * 128],
                            wo_r[:, kc, :],
                            start=(kc == 0),
                            stop=(kc == 3),
                        )
                    fst = fstage_pool.tile([128, DIM], F32, tag="fst")
                    cp(fst[:], psf[:])
                    row0 = rr * N + ic * 128
                    nc.sync.dma_start(out=out_ext[row0:row0 + 128, :], in_=fst[:])

        atp2_psum.release()
        fin_psum.release()
        av_psum.release()
        fstage_pool.release()
        oT_pool.release()
        attnT_pool.release()
        xT_pool.release()
        v_pool.release()
        consts.release()
        dram.release()

    if not nc.is_finalized():
        nc.finalize()
    return nc


_cache = {}


def _get_nc(scale: float):
    key = round(float(scale), 12)
    if key not in _cache:
        _cache[key] = build_nc(float(scale))
    return _cache[key]


def make_in_maps(x, Wq, Wkv, Wo):
    x = np.ascontiguousarray(np.asarray(x, dtype=np.float32)).reshape(CORES, RN, DIM)
    Wq = np.ascontiguousarray(np.asarray(Wq, dtype=np.float32))
    Wkv = np.asarray(Wkv, dtype=np.float32)
    Wk = np.ascontiguousarray(Wkv[:, :HD])
    Wv = np.ascontiguousarray(Wkv[:, HD:])
    Wo = np.ascontiguousarray(np.asarray(Wo, dtype=np.float32))
    return [
        {"x": x[c], "wq": Wq, "wk": Wk, "wv": Wv, "wo": Wo} for c in range(CORES)
    ]


def kernel(x, Wq, Wkv, Wo, bo, mask, tie_attn_dim):
    x = np.asarray(x)
    br, n, dim = x.shape
    r = int(tie_attn_dim)
    assert (br, n, dim) == (128, 512, 256) and r == 128, "kernel hardcodes shapes"
    mask = np.asarray(mask)
    assert mask.all(), "kernel assumes an all-valid mask"
    num_rows = float(mask.reshape(1, r, n).any(axis=-1).sum(axis=-1)[0])
    scale = (D ** -0.5) * (num_rows ** -0.5)

    nc = _get_nc(scale)
    in_maps = make_in_maps(x, Wq, Wkv, Wo)
    res = bass_utils.run_bass_kernel_spmd(nc, in_maps, core_ids=list(range(CORES)))
    out = np.concatenate([m["out"] for m in res.results], axis=0)
    out = out.reshape(br, n, dim)
    bo = np.asarray(bo, dtype=np.float32)
    if bo.any():
        out = out + bo
    return np.ascontiguousarray(out.astype(np.float32))

